# revision 3
# baseline (speedup 1.0000x reference)
"""Trainium2 Bass kernel v3 for vq_codebook (nn_BDFR_80925773791448).

Per core (SPMD x8): image b = core//2, own half = core%2. Full-image
redundant deviation map (no collectives).

Key layouts:
  h2   [128, 18, 4, 130] fp8: partition p = ch + 64*((hr//4)%2), free
       (j=hr//8, i=hr%4, col), hr = img_row + 8. j0/j17 + col 0/129 zero.
  projq[128, 16, 4, 2, 128] bf16: partition p = ch + 64*((r//4)%2),
       free (j=r//8, i=r%4, which: 0=proj 1=proj^2, pix).
All silu acts on the Act engine at 128 partitions. dwconv = 5 fp8
DoubleRow matmuls per row (2 arbitrary taps per matmul via ktile dim,
partition-half selected by weights; 8 row-phase weight variants).
"""

import numpy as np
import ml_dtypes

import concourse.bass as bass
import concourse.bacc as bacc
import concourse.tile as tile
import concourse.mybir as mybir
import concourse.bass_isa as bass_isa
from concourse.bass_utils import run_bass_kernel_spmd

F32 = mybir.dt.float32
BF16 = mybir.dt.bfloat16
FP8 = mybir.dt.float8e4
AF = mybir.ActivationFunctionType
OP = mybir.AluOpType
DR = mybir.MatmulPerfMode.DoubleRow

NPBF16 = ml_dtypes.bfloat16
NPFP8 = ml_dtypes.float8_e4m3fn

BN_EPS = 1e-5
TAU = 1.0

B, C, H, W = 4, 256, 128, 128
P, K = 64, 8
NCORES = 8
HC = 130
NJ = 18            # h2 j slots (hr = img_row + 8, hr in 0..143)
JSZ = 4 * HC       # elements per (partition, j slot)

# dwconv tap pairing: 5 streams x 2 ktiles (last ktile zero-weighted)
TAPS = [(-1, -1), (-1, 0), (-1, 1), (0, -1), (0, 0), (0, 1),
        (1, -1), (1, 0), (1, 1), None]

_CACHE = {}


def _bn_fold(p):
    g, b2, m, v = p[0], p[1], p[2], p[3]
    s = g / np.sqrt(v + BN_EPS)
    t = b2 - m * s
    return s, t


def _h2_off(r, dy, dx):
    """Element offset (within a partition) of h2[(ch-half for row r+dy)],
    free position for input row r+dy, col offset dx (+1 pad)."""
    hr = r + 8 + dy
    return (hr // 8) * JSZ + (hr % 4) * HC + (dx + 1)


def _h2_half(r, dy):
    return ((r + 8 + dy) // 4) % 2


def _prep_consts(inp):
    s1, t1 = _bn_fold(np.asarray(inp["fp_bn1"], np.float64))
    w1 = np.asarray(inp["fp_w1"], np.float64)[:, :, 0, 0]
    w1f = w1 * s1[:, None]
    w1d = np.empty((128, 2, 64), np.float64)
    for t in range(2):
        w1d[:, t, :] = w1f[:, 128 * t:128 * t + 128].T
    w1d = w1d.astype(NPFP8)

    s2, t2 = _bn_fold(np.asarray(inp["fp_bn2"], np.float64))
    dw = np.asarray(inp["fp_dw"], np.float64)[:, 0] * s2[:, None, None]
    # dw weights: [128, 2, 5, 8, 64] (ktile, stream, row-phase i8, out ch)
    dwv = np.zeros((128, 2, 5, 8, 64), np.float64)
    idx = np.arange(64)
    for i8 in range(8):
        for s in range(5):
            for t in range(2):
                tap = TAPS[2 * s + t]
                if tap is None:
                    continue
                dy, dx = tap
                half = _h2_half(i8, dy)
                dwv[64 * half + idx, t, s, i8, idx] = dw[:, dy + 1, dx + 1]
    dwv = dwv.astype(NPFP8)

    protos = np.asarray(inp["protos"], np.float64)
    drhs_a = (-2.0 * protos.T).astype(NPBF16)           # [64, 8]
    drhs_b = np.ones((64, K), NPBF16)
    pk2 = (protos * protos).sum(1).astype(np.float32).reshape(1, K)

    s_bs1, t_bs1 = _bn_fold(np.asarray(inp["bs_bn1"], np.float64))
    s_bl1, t_bl1 = _bn_fold(np.asarray(inp["bl_bn1"], np.float64))
    k3 = np.asarray(inp["bs_dw"], np.float64)[0, 0] * s_bs1[0]
    k5 = np.asarray(inp["bl_dw"], np.float64)[0, 0] * s_bl1[0]

    def bands(k, kw):
        r = kw // 2
        m = np.zeros((128, kw, 128), np.float64)
        for a in range(kw):
            for wout in range(128):
                for b2_ in range(kw):
                    win = wout + b2_ - r
                    if 0 <= win < 128:
                        m[win, a, wout] = k[a, b2_]
        return m.astype(NPBF16)

    b3w = bands(k3.T, 3)      # row-major bands: m[rin, dx, rout]
    b5w = bands(k5.T, 5)

    s_bs2, t_bs2 = _bn_fold(np.asarray(inp["bs_bn2"], np.float64))
    s_bl2, t_bl2 = _bn_fold(np.asarray(inp["bl_bn2"], np.float64))
    pw_s = np.asarray(inp["bs_pw"], np.float64)[:, 0, 0, 0]
    pw_l = np.asarray(inp["bl_pw"], np.float64)[:, 0, 0, 0]
    a_ch = np.concatenate([pw_s * s_bs2, pw_l * s_bl2])
    b_ch = np.concatenate([t_bs2, t_bl2])
    fw = np.asarray(inp["fuse_w"], np.float64)[0, :, 0, 0]
    fb = float(np.asarray(inp["fuse_b"], np.float64)[0])
    gamma = float(np.asarray(inp["gamma"], np.float64)[0])

    hb = np.zeros((1, 24), np.float32)
    hb[0, 0:16] = b_ch
    hb[0, 16] = t_bs1[0]
    hb[0, 17] = t_bl1[0]
    hb[0, 18] = fb / 2.0
    hb[0, 19] = 1.0
    hb[0, 20] = -1.0

    idm = np.eye(128, dtype=NPBF16)

    return dict(
        w1d=w1d, t1=np.tile(t1.astype(np.float32), 2).reshape(128, 1),
        dwv=dwv, t2=np.tile(t2.astype(np.float32), 2).reshape(128, 1),
        drhs_a=drhs_a, drhs_b=drhs_b, pk2=pk2,
        b3w=b3w, b5w=b5w, hb=hb, idm=idm,
        a_ch=a_ch, fw=fw, gamma=gamma,
        sc_d=float(1.0 / (TAU + 1e-6) ** 2),
    )


def _build_kernel(consts):
    nc = bacc.Bacc("TRN2", target_bir_lowering=False, num_devices=NCORES)

    xf8d = nc.declare_dram_parameter("xf8d", [128, 2, H, W], FP8, isOutput=False)
    xwmd = nc.declare_dram_parameter("xwmd", [128, C, 64], BF16, isOutput=False)
    w1d = nc.declare_dram_parameter("w1d", [128, 2, 64], FP8, isOutput=False)
    t1d = nc.declare_dram_parameter("t1d", [128, 1], F32, isOutput=False)
    dwvd = nc.declare_dram_parameter("dwvd", [128, 2, 5, 8, 64], FP8,
                                     isOutput=False)
    t2d = nc.declare_dram_parameter("t2d", [128, 1], F32, isOutput=False)
    drad = nc.declare_dram_parameter("drad", [64, K], BF16, isOutput=False)
    drbd = nc.declare_dram_parameter("drbd", [64, K], BF16, isOutput=False)
    pk2d = nc.declare_dram_parameter("pk2d", [1, K], F32, isOutput=False)
    b3wd = nc.declare_dram_parameter("b3wd", [128, 3, 128], BF16, isOutput=False)
    b5wd = nc.declare_dram_parameter("b5wd", [128, 5, 128], BF16, isOutput=False)
    hbd = nc.declare_dram_parameter("hbd", [1, 24], F32, isOutput=False)
    idmd = nc.declare_dram_parameter("idmd", [128, 128], BF16, isOutput=False)
    selmd = nc.declare_dram_parameter("selmd", [128, 64], BF16, isOutput=False)
    outd = nc.declare_dram_parameter("outd", [128, C, 64], BF16, isOutput=True)


    a_ch, fw = consts["a_ch"], consts["fw"]
    gamma = consts["gamma"]
    sc_d = consts["sc_d"]

    with tile.TileContext(nc) as tc:
        with (
            tc.tile_pool(name="const", bufs=1) as cpool,
            tc.tile_pool(name="xwm", bufs=1) as xwpool,
            tc.tile_pool(name="xs", bufs=3) as xspool,
            tc.tile_pool(name="hbuf", bufs=1) as hpool,
            tc.tile_pool(name="small", bufs=1) as spool,
            tc.tile_pool(name="uc", bufs=3) as ucpool,
            tc.tile_pool(name="outw", bufs=6) as opool,
            tc.tile_pool(name="ps_c", bufs=2, space="PSUM") as ps_c,  # [128,1024] tiles
            tc.tile_pool(name="ps_d", bufs=3, space="PSUM") as ps_d,
            tc.tile_pool(name="ps_s", bufs=1, space="PSUM") as ps_s,
        ):
            # ---- constants ----
            w1_s = cpool.tile([128, 2, 64], FP8)
            t1_s = cpool.tile([128, 1], F32)
            dwv_s = cpool.tile([128, 2, 5, 8, 64], FP8)
            t2_s = cpool.tile([128, 1], F32)
            dra_s = cpool.tile([64, K], BF16)
            drb_s = cpool.tile([64, K], BF16)
            pk2_s = cpool.tile([128, K], F32)
            b3w_s = cpool.tile([128, 3, 128], BF16)
            b5w_s = cpool.tile([128, 5, 128], BF16)
            hb_s = cpool.tile([128, 24], F32)
            idm_s = cpool.tile([128, 128], BF16)
            selm_s = cpool.tile([128, 64], BF16)
            nc.gpsimd.dma_start(out=w1_s[:], in_=w1d.ap())
            nc.gpsimd.dma_start(out=t1_s[:], in_=t1d.ap())
            nc.gpsimd.dma_start(out=dwv_s[:], in_=dwvd.ap())
            nc.gpsimd.dma_start(out=t2_s[:], in_=t2d.ap())

            # ---- persistent buffers ----
            xwm_s = xwpool.tile([128, C, 64], BF16)
            h2_s = hpool.tile([128, NJ, 4, HC], FP8)
            projq_s = hpool.tile([128, 16, 4, 2, W], BF16)
            md2_s = spool.tile([128, H], F32)
            md_s = spool.tile([128, H], BF16)
            dev_s = spool.tile([128, H + 4], BF16)
            mloc_s = spool.tile([128, 2], F32)
            mq_s = spool.tile([128, 2], F32)
            mmb_s = spool.tile([128, 2], F32)
            rec_s = spool.tile([128, 1], F32)
            scr_s = spool.tile([1, 1], F32)
            g1_s = spool.tile([128, H], F32)
            g2_s = spool.tile([128, H], F32)
            acc_s = spool.tile([128, H], F32)
            attnv_s = spool.tile([128, H], BF16)
            attnvh_s = spool.tile([128, 64], BF16)

            # zero pads: j0 and j17 slots of h2, cols 0/129, dev
            nc.vector.memset(h2_s[:, 0:NJ:NJ - 1, :, :], 0)
            nc.vector.memset(h2_s[:, :, :, 0:HC:HC - 1], 0)
            nc.vector.memset(dev_s[:], 0)

            # PE warmup
            warm = ps_s.tile([128, 512], F32, tag="sm", name="warm")
            nc.tensor.matmul(warm[0:64, 0:64], dwv_s[:, 0, 0, 0, :],
                             dwv_s[:, 0, 0, 0, :],
                             start=True, stop=True)
            del warm

            # xwm chunks interleaved into phase A below
            # ================= phase A =================
            xs_tiles = {}
            NS = 8

            def xs_load(g):
                if g >= NS:
                    return
                xt = xspool.tile([128, 2, 16, W], FP8, tag="xs",
                                 name=f"xs{g}")
                if g == 0:
                    nc.sync.dma_start(
                        out=xt[:, :, 0:8, :],
                        in_=xf8d.ap()[:, :, 0:8, :])
                    nc.sync.dma_start(
                        out=xt[:, :, 8:16, :],
                        in_=xf8d.ap()[:, :, 8:16, :])
                else:
                    nc.sync.dma_start(
                        out=xt[:], in_=xf8d.ap()[:, :, 16 * g:16 * g + 16, :])
                xs_tiles[g] = xt

            xs_load(0)
            xs_load(1)
            # late-needed consts (after the first x slabs)
            nc.sync.dma_start(out=dra_s[:], in_=drad.ap())
            nc.sync.dma_start(out=drb_s[:], in_=drbd.ap())
            pk2_b = bass.AP(tensor=pk2d.ap().tensor, offset=0,
                            ap=[[0, 128], [1, K]])
            nc.sync.dma_start(out=pk2_s[:], in_=pk2_b)
            nc.gpsimd.dma_start(out=b3w_s[:], in_=b3wd.ap())
            nc.gpsimd.dma_start(out=b5w_s[:], in_=b5wd.ap())
            hb_b = bass.AP(tensor=hbd.ap().tensor, offset=0,
                           ap=[[0, 128], [1, 24]])
            nc.gpsimd.dma_start(out=hb_s[:], in_=hb_b)
            nc.sync.dma_start(out=idm_s[:], in_=idmd.ap())
            nc.sync.dma_start(out=selm_s[:], in_=selmd.ap())

            psd_groups = {}
            h2_base = h2_s[:, 0, 0, 0]
            h2_pdim = list(h2_base.ap[0])
            h2_off0 = h2_base.offset

            def dwconv_16rows(gd):
                p0 = 16 * gd
                for kk in range(2):          # 8-row halves -> psum banks
                    psd = ps_d.tile([128, 512], F32, tag="dw",
                                    name=f"psd{gd}_{kk}")
                    for ri in range(8):
                        r = p0 + 8 * kk + ri
                        i8 = r % 8
                        half = (ri // 4) % 2
                        po = (psd[0:64, 128 * (ri % 4):128 * (ri % 4) + 128]
                              if half == 0 else
                              psd[64:128, 128 * (ri % 4):128 * (ri % 4) + 128])
                        tp = (0, 0) if half == 0 else (0, 64)
                        for s in range(5):
                            t0 = TAPS[2 * s]
                            t1_ = TAPS[2 * s + 1]
                            o0 = _h2_off(r, t0[0], t0[1])
                            kts = (_h2_off(r, t1_[0], t1_[1]) - o0
                                   if t1_ is not None else 0)
                            rhs = bass.AP(
                                tensor=h2_base.tensor,
                                offset=h2_off0 + o0,
                                ap=[h2_pdim, [kts, 2], [1, W]])
                            nc.tensor.matmul(
                                po, dwv_s[:, :, s, i8, :], rhs,
                                start=(s == 0), stop=(s == 4),
                                perf_mode=DR, tile_position=tp)
                    # silu2: [128, 512] -> projq[:, j, :, 0, :]
                    j = 2 * gd + kk
                    nc.scalar.activation(
                        out=projq_s[:, j, :, 0, :],
                        in_=psd[:].rearrange("p (a b) -> p a b", a=4),
                        func=AF.Silu, bias=t2_s[:], scale=1.0)
                    del psd
                # square on DVE (bf16 2x)
                j0 = 2 * gd
                nc.vector.tensor_tensor(
                    out=projq_s[:, j0:j0 + 2, :, 1, :],
                    in0=projq_s[:, j0:j0 + 2, :, 0, :],
                    in1=projq_s[:, j0:j0 + 2, :, 0, :], op=OP.mult)
                # distance matmuls for rows p0..p0+15
                g2i = gd // 4
                if g2i not in psd_groups:
                    psd_groups[g2i] = ps_s.tile(
                        [128, 512], F32, tag="sm", name=f"dist{g2i}")
                pg = psd_groups[g2i]
                for r in range(p0, p0 + 16):
                    rr = r - 64 * g2i
                    j, i, hf = r // 8, r % 4, (r // 4) % 2
                    lt0 = projq_s[64 * hf:64 * hf + 64, j, i, 0, :]
                    lt1 = projq_s[64 * hf:64 * hf + 64, j, i, 1, :]
                    nc.tensor.matmul(pg[:, K * rr:K * rr + K], lt0,
                                     dra_s[:], start=True, stop=False,
                                     tile_position=(0, 0))
                    nc.tensor.matmul(pg[:, K * rr:K * rr + K], lt1,
                                     drb_s[:], start=False, stop=True,
                                     tile_position=(0, 0))
                rr0 = p0 - 64 * g2i
                pk2b = bass.AP(
                    tensor=pk2_s[:].tensor, offset=0,
                    ap=[list(pk2_s[:].ap[0]), [0, 16], [1, K]])
                pslice = pg[:, K * rr0:K * rr0 + 16 * K]
                nc.vector.tensor_tensor(
                    out=pslice, in0=pslice, in1=pk2b, op=OP.add)
                nc.vector.tensor_reduce(
                    out=md2_s[:, p0:p0 + 16],
                    in_=pslice.rearrange("p (a b) -> p a b", b=K),
                    axis=mybir.AxisListType.X, op=OP.min)
                if gd % 4 == 3:
                    del psd_groups[g2i]

            for g in range(NS):
                xs_load(g + 2)
                xt = xs_tiles[g]
                # conv1: one [128, 1024] psum tile per 16 rows
                psc = ps_c.tile([128, 1024], F32, tag="c1", name=f"psc{g}")
                for j4 in range(4):
                    tp = (0, 0) if j4 % 2 == 0 else (0, 64)
                    bank = j4 // 2
                    po = psc[64 * (j4 % 2):64 * (j4 % 2) + 64,
                             512 * bank:512 * bank + 512]
                    rbase = 4 * j4
                    nc.tensor.matmul(
                        po, w1_s[:],
                        xt[:, :, rbase:rbase + 4, :].rearrange(
                            "p t a b -> p t (a b)"),
                        start=True, stop=True, perf_mode=DR,
                        tile_position=tp)
                # silu1 -> h2 j slots 2g+1, 2g+2
                nc.scalar.activation(
                    out=h2_s[:, 2 * g + 1:2 * g + 3, :, 1:W + 1],
                    in_=psc[:].rearrange("p (a c b) -> p a c b", a=2, c=4),
                    func=AF.Silu, bias=t1_s[:], scale=1.0)
                del psc
                del xs_tiles[g]
                # xwm chunk (Pool queue)
                c0x = 32 * g
                nc.gpsimd.dma_start(out=xwm_s[:, c0x:c0x + 32, :],
                                    in_=xwmd.ap()[:, c0x:c0x + 32, :])
                if g > 0:
                    dwconv_16rows(g - 1)
                if g == NS - 1:
                    dwconv_16rows(NS - 1)

            # ================= minmax + dev (all in u = md space) ======
            nc.vector.tensor_scalar(out=md2_s[:], in0=md2_s[:], scalar1=0.0,
                                    scalar2=None, op0=OP.max)
            nc.scalar.activation(out=md_s[:], in_=md2_s[:], func=AF.Sqrt,
                                 scale=sc_d)
            # dummy act to pull the silu table back in before the head
            nc.scalar.activation(out=scr_s[0:1, 0:1], in_=hb_s[0:1, 19:20],
                                 func=AF.Silu, bias=hb_s[0:1, 21:22],
                                 scale=0.0)
            nc.vector.tensor_reduce(out=mloc_s[:, 0:1], in_=md_s[:],
                                    axis=mybir.AxisListType.X, op=OP.max)
            nc.vector.tensor_reduce(out=mloc_s[:, 1:2], in_=md_s[:],
                                    axis=mybir.AxisListType.X, op=OP.min)
            nc.vector.tensor_scalar(out=mloc_s[:, 1:2], in0=mloc_s[:, 1:2],
                                    scalar1=-1.0, scalar2=None, op0=OP.mult)
            mdT = ps_s.tile([128, 128], BF16, tag="sm", name="mdT")
            nc.tensor.transpose(mdT[:, :], md_s[:], idm_s[:])
            nc.gpsimd.partition_all_reduce(
                mq_s[:], mloc_s[:], 128, bass_isa.ReduceOp.max)
            # mmb = [dmax, dmin] (negate col 1 back)
            nc.vector.tensor_tensor(out=mmb_s[:], in0=mq_s[:],
                                    in1=hb_s[:, 19:21], op=OP.mult)
            nc.vector.tensor_tensor(out=rec_s[:], in0=mmb_s[:, 0:1],
                                    in1=mmb_s[:, 1:2], op=OP.subtract)
            nc.vector.tensor_scalar(out=rec_s[:], in0=rec_s[:], scalar1=1e-6,
                                    scalar2=None, op0=OP.add)
            nc.vector.reciprocal(out=rec_s[:], in_=rec_s[:])
            nc.vector.tensor_scalar(out=dev_s[:, 2:H + 2], in0=mdT[:],
                                    scalar1=mmb_s[:, 1:2],
                                    scalar2=rec_s[:],
                                    op0=OP.subtract, op1=OP.mult)
            del mdT

            # ================= head (W-major, full rows) =================
            ph1 = ps_s.tile([128, 128], F32, tag="sm", name="ph1")
            for a in range(3):
                nc.tensor.matmul(ph1[:, :], b3w_s[:, a, :],
                                 dev_s[:, 1 + a:1 + a + H],
                                 start=(a == 0), stop=(a == 2))
            nc.scalar.activation(out=g1_s[:], in_=ph1[:], func=AF.Silu,
                                 bias=hb_s[:, 16:17], scale=1.0)
            del ph1
            ph2 = ps_s.tile([128, 128], F32, tag="sm", name="ph2")
            for a in range(5):
                nc.tensor.matmul(ph2[:, :], b5w_s[:, a, :],
                                 dev_s[:, a:a + H],
                                 start=(a == 0), stop=(a == 4))
            nc.scalar.activation(out=g2_s[:], in_=ph2[:], func=AF.Silu,
                                 bias=hb_s[:, 17:18], scale=1.0)
            del ph2
            uc = ucpool.tile([128, H], F32, tag="uc")
            nc.scalar.activation(out=uc[:], in_=g1_s[:], func=AF.Silu,
                                 bias=hb_s[:, 0:1], scale=float(a_ch[0]))
            nc.vector.tensor_scalar(out=acc_s[:], in0=uc[:],
                                    scalar1=float(fw[0]), scalar2=None,
                                    op0=OP.mult)
            for ch in range(1, 16):
                src = g1_s if ch < 8 else g2_s
                uc = ucpool.tile([128, H], F32, tag="uc")
                nc.scalar.activation(out=uc[:], in_=src[:], func=AF.Silu,
                                     bias=hb_s[:, ch:ch + 1],
                                     scale=float(a_ch[ch]))
                nc.vector.scalar_tensor_tensor(
                    out=acc_s[:], in0=uc[:], scalar=float(fw[ch]),
                    in1=acc_s[:], op0=OP.mult, op1=OP.add)
            nc.scalar.activation(out=acc_s[:], in_=acc_s[:], func=AF.Tanh,
                                 bias=hb_s[:, 18:19], scale=0.5)
            nc.vector.tensor_scalar(out=attnv_s[:], in0=acc_s[:],
                                    scalar1=gamma / 2.0,
                                    scalar2=1.0 + gamma / 2.0,
                                    op0=OP.mult, op1=OP.add)
            psel = ps_s.tile([128, 64], F32, tag="sm", name="psel")
            nc.tensor.matmul(psel[:, :], attnv_s[:], selm_s[:],
                             start=True, stop=True)
            nc.vector.tensor_copy(out=attnvh_s[:], in_=psel[:])
            del psel

            # ================= final =================
            dmaq = [nc.sync, nc.scalar, nc.gpsimd, nc.scalar,
                    nc.sync, nc.scalar, nc.gpsimd, nc.sync,
                    nc.scalar, nc.gpsimd, nc.sync, nc.scalar,
                    nc.sync, nc.gpsimd, nc.scalar, nc.sync]
            pool_tt = {2, 5, 8, 11}
            for i in range(16):
                c0 = 16 * i
                ot = opool.tile([128, 16, 64], BF16, tag="out",
                                name=f"ot{i}")
                av = attnvh_s[:]
                a_b = bass.AP(tensor=av.tensor, offset=av.offset,
                              ap=[list(av.ap[0]), [0, 16], [1, 64]])
                eng = nc.gpsimd if i in pool_tt else nc.vector
                eng.tensor_tensor(out=ot[:], in0=xwm_s[:, c0:c0 + 16, :],
                                  in1=a_b, op=OP.mult)
                dmaq[i].dma_start(out=outd.ap()[:, c0:c0 + 16, :],
                                  in_=ot[:])

    nc.compile()
    return nc


def _shard_inputs(inp, consts):
    x = np.asarray(inp["x"], np.float32)
    in_maps = []
    for j in range(NCORES):
        b, half = j // 2, j % 2
        r0 = 64 * half
        xb = x[b]
        xf8 = np.ascontiguousarray(
            xb.reshape(2, 128, H, W).transpose(1, 0, 2, 3)).astype(NPFP8)
        xwm = np.ascontiguousarray(
            xb[:, r0:r0 + 64, :].transpose(2, 0, 1)).astype(NPBF16)
        selm = np.zeros((128, 64), NPBF16)
        selm[r0 + np.arange(64), np.arange(64)] = 1
        in_maps.append({
            "xf8d": xf8, "xwmd": xwm, "selmd": selm,
            "w1d": consts["w1d"], "t1d": consts["t1"],
            "dwvd": consts["dwv"], "t2d": consts["t2"],
            "drad": consts["drhs_a"], "drbd": consts["drhs_b"],
            "pk2d": consts["pk2"],
            "b3wd": consts["b3w"], "b5wd": consts["b5w"],
            "hbd": consts["hb"], "idmd": consts["idm"],
        })
    return in_maps


def kernel(**inputs) -> np.ndarray:
    consts = _prep_consts(inputs)
    key = "nc"
    if key not in _CACHE:
        _CACHE[key] = _build_kernel(consts)
    nc = _CACHE[key]
    in_maps = _shard_inputs(inputs, consts)
    res = run_bass_kernel_spmd(nc, in_maps, list(range(NCORES)))
    out = np.empty((B, C, H, W), np.float32)
    for j in range(NCORES):
        b, half = j // 2, j % 2
        shard = np.asarray(res.results[j]["outd"]).astype(np.float32)
        out[b, :, 64 * half:64 * half + 64, :] = shard.transpose(1, 2, 0)
    return out


# revision 4
# speedup vs baseline: 1.0348x; 1.0348x over previous
"""Trainium2 Bass kernel v3 for vq_codebook (nn_BDFR_80925773791448).

Per core (SPMD x8): image b = core//2, own half = core%2. Full-image
redundant deviation map (no collectives).

Key layouts:
  h2   [128, 18, 4, 130] fp8: partition p = ch + 64*((hr//4)%2), free
       (j=hr//8, i=hr%4, col), hr = img_row + 8. j0/j17 + col 0/129 zero.
  projq[128, 16, 4, 2, 128] bf16: partition p = ch + 64*((r//4)%2),
       free (j=r//8, i=r%4, which: 0=proj 1=proj^2, pix).
All silu acts on the Act engine at 128 partitions. dwconv = 5 fp8
DoubleRow matmuls per row (2 arbitrary taps per matmul via ktile dim,
partition-half selected by weights; 8 row-phase weight variants).
"""

import numpy as np
import ml_dtypes

import concourse.bass as bass
import concourse.bacc as bacc
import concourse.tile as tile
import concourse.mybir as mybir
import concourse.bass_isa as bass_isa
from concourse.bass_utils import run_bass_kernel_spmd

F32 = mybir.dt.float32
BF16 = mybir.dt.bfloat16
FP8 = mybir.dt.float8e4
AF = mybir.ActivationFunctionType
OP = mybir.AluOpType
DR = mybir.MatmulPerfMode.DoubleRow

NPBF16 = ml_dtypes.bfloat16
NPFP8 = ml_dtypes.float8_e4m3fn

BN_EPS = 1e-5
TAU = 1.0

B, C, H, W = 4, 256, 128, 128
P, K = 64, 8
NCORES = 8
HC = 130
NJ = 18            # h2 j slots (hr = img_row + 8, hr in 0..143)
JSZ = 4 * HC       # elements per (partition, j slot)

# dwconv tap pairing: 5 streams x 2 ktiles (last ktile zero-weighted)
TAPS = [(-1, -1), (-1, 0), (-1, 1), (0, -1), (0, 0), (0, 1),
        (1, -1), (1, 0), (1, 1), None]

_CACHE = {}


def _bn_fold(p):
    g, b2, m, v = p[0], p[1], p[2], p[3]
    s = g / np.sqrt(v + BN_EPS)
    t = b2 - m * s
    return s, t


def _h2_off(r, dy, dx):
    """Element offset (within a partition) of h2[(ch-half for row r+dy)],
    free position for input row r+dy, col offset dx (+1 pad)."""
    hr = r + 8 + dy
    return (hr // 8) * JSZ + (hr % 4) * HC + (dx + 1)


def _h2_half(r, dy):
    return ((r + 8 + dy) // 4) % 2


def _prep_consts(inp):
    s1, t1 = _bn_fold(np.asarray(inp["fp_bn1"], np.float64))
    w1 = np.asarray(inp["fp_w1"], np.float64)[:, :, 0, 0]
    w1f = w1 * s1[:, None]
    w1d = np.empty((128, 2, 64), np.float64)
    for t in range(2):
        w1d[:, t, :] = w1f[:, 128 * t:128 * t + 128].T
    w1d = w1d.astype(NPFP8)

    s2, t2 = _bn_fold(np.asarray(inp["fp_bn2"], np.float64))
    dw = np.asarray(inp["fp_dw"], np.float64)[:, 0] * s2[:, None, None]
    # dw weights: [128, 2, 5, 8, 64] (ktile, stream, row-phase i8, out ch)
    dwv = np.zeros((128, 2, 5, 8, 64), np.float64)
    idx = np.arange(64)
    for i8 in range(8):
        for s in range(5):
            for t in range(2):
                tap = TAPS[2 * s + t]
                if tap is None:
                    continue
                dy, dx = tap
                half = _h2_half(i8, dy)
                dwv[64 * half + idx, t, s, i8, idx] = dw[:, dy + 1, dx + 1]
    dwv = dwv.astype(NPFP8)

    protos = np.asarray(inp["protos"], np.float64)
    drhs_a = np.tile((-2.0 * protos.T), (2, 1)).astype(NPBF16)   # [128, 8]
    drhs_b = np.ones((128, K), NPBF16)
    pk2 = (protos * protos).sum(1).astype(np.float32).reshape(1, K)

    s_bs1, t_bs1 = _bn_fold(np.asarray(inp["bs_bn1"], np.float64))
    s_bl1, t_bl1 = _bn_fold(np.asarray(inp["bl_bn1"], np.float64))
    k3 = np.asarray(inp["bs_dw"], np.float64)[0, 0] * s_bs1[0]
    k5 = np.asarray(inp["bl_dw"], np.float64)[0, 0] * s_bl1[0]

    def bands(k, kw):
        r = kw // 2
        m = np.zeros((128, kw, 128), np.float64)
        for a in range(kw):
            for wout in range(128):
                for b2_ in range(kw):
                    win = wout + b2_ - r
                    if 0 <= win < 128:
                        m[win, a, wout] = k[a, b2_]
        return m.astype(NPBF16)

    b3w = bands(k3.T, 3)      # row-major bands: m[rin, dx, rout]
    b5w = bands(k5.T, 5)

    s_bs2, t_bs2 = _bn_fold(np.asarray(inp["bs_bn2"], np.float64))
    s_bl2, t_bl2 = _bn_fold(np.asarray(inp["bl_bn2"], np.float64))
    pw_s = np.asarray(inp["bs_pw"], np.float64)[:, 0, 0, 0]
    pw_l = np.asarray(inp["bl_pw"], np.float64)[:, 0, 0, 0]
    a_ch = np.concatenate([pw_s * s_bs2, pw_l * s_bl2])
    b_ch = np.concatenate([t_bs2, t_bl2])
    fw = np.asarray(inp["fuse_w"], np.float64)[0, :, 0, 0]
    fb = float(np.asarray(inp["fuse_b"], np.float64)[0])
    gamma = float(np.asarray(inp["gamma"], np.float64)[0])

    hb = np.zeros((1, 24), np.float32)
    hb[0, 0:16] = b_ch
    hb[0, 16] = t_bs1[0]
    hb[0, 17] = t_bl1[0]
    hb[0, 18] = fb / 2.0
    hb[0, 19] = 1.0
    hb[0, 20] = -1.0

    idm = np.eye(128, dtype=NPBF16)

    return dict(
        w1d=w1d, t1=np.tile(t1.astype(np.float32), 2).reshape(128, 1),
        dwv=dwv, t2=np.tile(t2.astype(np.float32), 2).reshape(128, 1),
        drhs_a=drhs_a, drhs_b=drhs_b, pk2=pk2,
        b3w=b3w, b5w=b5w, hb=hb, idm=idm,
        a_ch=a_ch, fw=fw, gamma=gamma,
        sc_d=float(1.0 / (TAU + 1e-6) ** 2),
    )


def _build_kernel(consts):
    nc = bacc.Bacc("TRN2", target_bir_lowering=False, num_devices=NCORES)

    xf8d = nc.declare_dram_parameter("xf8d", [128, 2, H, W], FP8, isOutput=False)
    xwmd = nc.declare_dram_parameter("xwmd", [128, C, 64], BF16, isOutput=False)
    w1d = nc.declare_dram_parameter("w1d", [128, 2, 64], FP8, isOutput=False)
    t1d = nc.declare_dram_parameter("t1d", [128, 1], F32, isOutput=False)
    dwvd = nc.declare_dram_parameter("dwvd", [128, 2, 5, 8, 64], FP8,
                                     isOutput=False)
    t2d = nc.declare_dram_parameter("t2d", [128, 1], F32, isOutput=False)
    drad = nc.declare_dram_parameter("drad", [128, K], BF16, isOutput=False)
    drbd = nc.declare_dram_parameter("drbd", [128, K], BF16, isOutput=False)
    pk2d = nc.declare_dram_parameter("pk2d", [1, K], F32, isOutput=False)
    b3wd = nc.declare_dram_parameter("b3wd", [128, 3, 128], BF16, isOutput=False)
    b5wd = nc.declare_dram_parameter("b5wd", [128, 5, 128], BF16, isOutput=False)
    hbd = nc.declare_dram_parameter("hbd", [1, 24], F32, isOutput=False)
    idmd = nc.declare_dram_parameter("idmd", [128, 128], BF16, isOutput=False)
    selmd = nc.declare_dram_parameter("selmd", [128, 64], BF16, isOutput=False)
    outd = nc.declare_dram_parameter("outd", [128, C, 64], BF16, isOutput=True)


    a_ch, fw = consts["a_ch"], consts["fw"]
    gamma = consts["gamma"]
    sc_d = consts["sc_d"]

    with tile.TileContext(nc) as tc:
        with (
            tc.tile_pool(name="const", bufs=1) as cpool,
            tc.tile_pool(name="xwm", bufs=1) as xwpool,
            tc.tile_pool(name="xs", bufs=3) as xspool,
            tc.tile_pool(name="hbuf", bufs=1) as hpool,
            tc.tile_pool(name="small", bufs=1) as spool,
            tc.tile_pool(name="uc", bufs=3) as ucpool,
            tc.tile_pool(name="outw", bufs=6) as opool,
            tc.tile_pool(name="ps_c", bufs=2, space="PSUM") as ps_c,  # [128,1024] tiles
            tc.tile_pool(name="ps_d", bufs=3, space="PSUM") as ps_d,
            tc.tile_pool(name="ps_s", bufs=1, space="PSUM") as ps_s,
        ):
            # ---- constants ----
            w1_s = cpool.tile([128, 2, 64], FP8)
            t1_s = cpool.tile([128, 1], F32)
            dwv_s = cpool.tile([128, 2, 5, 8, 64], FP8)
            t2_s = cpool.tile([128, 1], F32)
            dra_s = cpool.tile([128, K], BF16)
            drb_s = cpool.tile([128, K], BF16)
            pk2_s = cpool.tile([128, K], F32)
            b3w_s = cpool.tile([128, 3, 128], BF16)
            b5w_s = cpool.tile([128, 5, 128], BF16)
            hb_s = cpool.tile([128, 24], F32)
            idm_s = cpool.tile([128, 128], BF16)
            selm_s = cpool.tile([128, 64], BF16)
            nc.gpsimd.dma_start(out=w1_s[:], in_=w1d.ap())
            nc.gpsimd.dma_start(out=t1_s[:], in_=t1d.ap())
            nc.gpsimd.dma_start(out=dwv_s[:], in_=dwvd.ap())
            nc.gpsimd.dma_start(out=t2_s[:], in_=t2d.ap())

            # ---- persistent buffers ----
            xwm_s = xwpool.tile([128, C, 64], BF16)
            h2_s = hpool.tile([128, NJ, 4, HC], FP8)
            projq_s = hpool.tile([128, 16, 4, 2, W], BF16)
            md2_s = spool.tile([128, H], F32)
            md_s = spool.tile([128, H], BF16)
            dev_s = spool.tile([128, H + 4], BF16)
            mloc_s = spool.tile([128, 2], F32)
            mq_s = spool.tile([128, 2], F32)
            mmb_s = spool.tile([128, 2], F32)
            rec_s = spool.tile([128, 1], F32)
            scr_s = spool.tile([1, 1], F32)
            g1_s = spool.tile([128, H], F32)
            g2_s = spool.tile([128, H], F32)
            acc_s = spool.tile([128, H], F32)
            attnv_s = spool.tile([128, H], BF16)
            attnvh_s = spool.tile([128, 64], BF16)

            # zero pads: j0 and j17 slots of h2, cols 0/129, dev
            nc.vector.memset(h2_s[:, 0:NJ:NJ - 1, :, :], 0)
            nc.vector.memset(h2_s[:, :, :, 0:HC:HC - 1], 0)
            nc.vector.memset(dev_s[:], 0)

            # PE warmup
            warm = ps_s.tile([128, 512], F32, tag="sm", name="warm")
            nc.tensor.matmul(warm[0:64, 0:64], dwv_s[:, 0, 0, 0, :],
                             dwv_s[:, 0, 0, 0, :],
                             start=True, stop=True)
            del warm

            # xwm chunks interleaved into phase A below
            # ================= phase A =================
            xs_tiles = {}
            NS = 8

            def xs_load(g):
                if g >= NS:
                    return
                xt = xspool.tile([128, 2, 16, W], FP8, tag="xs",
                                 name=f"xs{g}")
                if g == 0:
                    nc.sync.dma_start(
                        out=xt[:, :, 0:8, :],
                        in_=xf8d.ap()[:, :, 0:8, :])
                    nc.sync.dma_start(
                        out=xt[:, :, 8:16, :],
                        in_=xf8d.ap()[:, :, 8:16, :])
                else:
                    nc.sync.dma_start(
                        out=xt[:], in_=xf8d.ap()[:, :, 16 * g:16 * g + 16, :])
                xs_tiles[g] = xt

            xs_load(0)
            xs_load(1)
            # late-needed consts (after the first x slabs)
            nc.sync.dma_start(out=dra_s[:], in_=drad.ap())
            nc.sync.dma_start(out=drb_s[:], in_=drbd.ap())
            pk2_b = bass.AP(tensor=pk2d.ap().tensor, offset=0,
                            ap=[[0, 128], [1, K]])
            nc.sync.dma_start(out=pk2_s[:], in_=pk2_b)
            nc.gpsimd.dma_start(out=b3w_s[:], in_=b3wd.ap())
            nc.gpsimd.dma_start(out=b5w_s[:], in_=b5wd.ap())
            hb_b = bass.AP(tensor=hbd.ap().tensor, offset=0,
                           ap=[[0, 128], [1, 24]])
            nc.gpsimd.dma_start(out=hb_s[:], in_=hb_b)
            nc.sync.dma_start(out=idm_s[:], in_=idmd.ap())
            nc.sync.dma_start(out=selm_s[:], in_=selmd.ap())

            psd_groups = {}
            h2_base = h2_s[:, 0, 0, 0]
            h2_pdim = list(h2_base.ap[0])
            h2_off0 = h2_base.offset

            def dwconv_16rows(gd):
                p0 = 16 * gd
                for kk in range(2):          # 8-row halves -> psum banks
                    psd = ps_d.tile([128, 512], F32, tag="dw",
                                    name=f"psd{gd}_{kk}")
                    for ri in range(8):
                        r = p0 + 8 * kk + ri
                        i8 = r % 8
                        half = (ri // 4) % 2
                        po = (psd[0:64, 128 * (ri % 4):128 * (ri % 4) + 128]
                              if half == 0 else
                              psd[64:128, 128 * (ri % 4):128 * (ri % 4) + 128])
                        tp = (0, 0) if half == 0 else (0, 64)
                        for s in range(5):
                            t0 = TAPS[2 * s]
                            t1_ = TAPS[2 * s + 1]
                            o0 = _h2_off(r, t0[0], t0[1])
                            kts = (_h2_off(r, t1_[0], t1_[1]) - o0
                                   if t1_ is not None else 0)
                            rhs = bass.AP(
                                tensor=h2_base.tensor,
                                offset=h2_off0 + o0,
                                ap=[h2_pdim, [kts, 2], [1, W]])
                            nc.tensor.matmul(
                                po, dwv_s[:, :, s, i8, :], rhs,
                                start=(s == 0), stop=(s == 4),
                                perf_mode=DR, tile_position=tp)
                    # silu2: [128, 512] -> projq[:, j, :, 0, :]
                    j = 2 * gd + kk
                    nc.scalar.activation(
                        out=projq_s[:, j, :, 0, :],
                        in_=psd[:].rearrange("p (a b) -> p a b", a=4),
                        func=AF.Silu, bias=t2_s[:], scale=1.0)
                    del psd
                # square on DVE (bf16 2x)
                j0 = 2 * gd
                nc.vector.tensor_tensor(
                    out=projq_s[:, j0:j0 + 2, :, 1, :],
                    in0=projq_s[:, j0:j0 + 2, :, 0, :],
                    in1=projq_s[:, j0:j0 + 2, :, 0, :], op=OP.mult)
                # distance matmuls for rows p0..p0+15
                g2i = gd // 4
                if g2i not in psd_groups:
                    psd_groups[g2i] = ps_s.tile(
                        [128, 512], F32, tag="sm", name=f"dist{g2i}")
                pg = psd_groups[g2i]
                for r in range(p0, p0 + 16):
                    rr = r - 64 * g2i
                    j, i, hf = r // 8, r % 4, (r // 4) % 2
                    lt0 = projq_s[64 * hf:64 * hf + 64, j, i, 0, :]
                    lt1 = projq_s[64 * hf:64 * hf + 64, j, i, 1, :]
                    ra = dra_s[64 * hf:64 * hf + 64, :]
                    rb = drb_s[64 * hf:64 * hf + 64, :]
                    nc.tensor.matmul(pg[:, K * rr:K * rr + K], lt0,
                                     ra, start=True, stop=False,
                                     tile_position=(64 * hf, 0))
                    nc.tensor.matmul(pg[:, K * rr:K * rr + K], lt1,
                                     rb, start=False, stop=True,
                                     tile_position=(64 * hf, 0))
                rr0 = p0 - 64 * g2i
                pk2b = bass.AP(
                    tensor=pk2_s[:].tensor, offset=0,
                    ap=[list(pk2_s[:].ap[0]), [0, 16], [1, K]])
                pslice = pg[:, K * rr0:K * rr0 + 16 * K]
                nc.vector.tensor_tensor(
                    out=pslice, in0=pslice, in1=pk2b, op=OP.add)
                nc.vector.tensor_reduce(
                    out=md2_s[:, p0:p0 + 16],
                    in_=pslice.rearrange("p (a b) -> p a b", b=K),
                    axis=mybir.AxisListType.X, op=OP.min)
                if gd % 4 == 3:
                    del psd_groups[g2i]

            for g in range(NS):
                xs_load(g + 2)
                xt = xs_tiles[g]
                # conv1: one [128, 1024] psum tile per 16 rows
                psc = ps_c.tile([128, 1024], F32, tag="c1", name=f"psc{g}")
                for j4 in range(4):
                    tp = (0, 0) if j4 % 2 == 0 else (0, 64)
                    bank = j4 // 2
                    po = psc[64 * (j4 % 2):64 * (j4 % 2) + 64,
                             512 * bank:512 * bank + 512]
                    rbase = 4 * j4
                    nc.tensor.matmul(
                        po, w1_s[:],
                        xt[:, :, rbase:rbase + 4, :].rearrange(
                            "p t a b -> p t (a b)"),
                        start=True, stop=True, perf_mode=DR,
                        tile_position=tp)
                # silu1 -> h2 j slots 2g+1, 2g+2
                nc.scalar.activation(
                    out=h2_s[:, 2 * g + 1:2 * g + 3, :, 1:W + 1],
                    in_=psc[:].rearrange("p (a c b) -> p a c b", a=2, c=4),
                    func=AF.Silu, bias=t1_s[:], scale=1.0)
                del psc
                del xs_tiles[g]
                # xwm chunk (Pool queue)
                c0x = 32 * g
                nc.gpsimd.dma_start(out=xwm_s[:, c0x:c0x + 32, :],
                                    in_=xwmd.ap()[:, c0x:c0x + 32, :])
                if g > 0:
                    dwconv_16rows(g - 1)
                if g == NS - 1:
                    dwconv_16rows(NS - 1)

            # ================= minmax + dev (all in u = md space) ======
            nc.vector.tensor_scalar(out=md2_s[:], in0=md2_s[:], scalar1=0.0,
                                    scalar2=None, op0=OP.max)
            nc.scalar.activation(out=md_s[:], in_=md2_s[:], func=AF.Sqrt,
                                 scale=sc_d)
            # dummy act to pull the silu table back in before the head
            nc.scalar.activation(out=scr_s[0:1, 0:1], in_=hb_s[0:1, 19:20],
                                 func=AF.Silu, bias=hb_s[0:1, 21:22],
                                 scale=0.0)
            nc.vector.tensor_reduce(out=mloc_s[:, 0:1], in_=md_s[:],
                                    axis=mybir.AxisListType.X, op=OP.max)
            nc.vector.tensor_reduce(out=mloc_s[:, 1:2], in_=md_s[:],
                                    axis=mybir.AxisListType.X, op=OP.min)
            nc.vector.tensor_scalar(out=mloc_s[:, 1:2], in0=mloc_s[:, 1:2],
                                    scalar1=-1.0, scalar2=None, op0=OP.mult)
            mdT = ps_s.tile([128, 128], BF16, tag="sm", name="mdT")
            nc.tensor.transpose(mdT[:, :], md_s[:], idm_s[:])
            nc.gpsimd.partition_all_reduce(
                mq_s[:], mloc_s[:], 128, bass_isa.ReduceOp.max)
            # mmb = [dmax, dmin] (negate col 1 back)
            nc.vector.tensor_tensor(out=mmb_s[:], in0=mq_s[:],
                                    in1=hb_s[:, 19:21], op=OP.mult)
            nc.vector.tensor_tensor(out=rec_s[:], in0=mmb_s[:, 0:1],
                                    in1=mmb_s[:, 1:2], op=OP.subtract)
            nc.vector.tensor_scalar(out=rec_s[:], in0=rec_s[:], scalar1=1e-6,
                                    scalar2=None, op0=OP.add)
            nc.vector.reciprocal(out=rec_s[:], in_=rec_s[:])
            nc.vector.tensor_scalar(out=dev_s[:, 2:H + 2], in0=mdT[:],
                                    scalar1=mmb_s[:, 1:2],
                                    scalar2=rec_s[:],
                                    op0=OP.subtract, op1=OP.mult)
            del mdT

            # ================= head (W-major, full rows) =================
            ph1 = ps_s.tile([128, 128], F32, tag="sm", name="ph1")
            for a in range(3):
                nc.tensor.matmul(ph1[:, :], b3w_s[:, a, :],
                                 dev_s[:, 1 + a:1 + a + H],
                                 start=(a == 0), stop=(a == 2))
            nc.scalar.activation(out=g1_s[:], in_=ph1[:], func=AF.Silu,
                                 bias=hb_s[:, 16:17], scale=1.0)
            del ph1
            ph2 = ps_s.tile([128, 128], F32, tag="sm", name="ph2")
            for a in range(5):
                nc.tensor.matmul(ph2[:, :], b5w_s[:, a, :],
                                 dev_s[:, a:a + H],
                                 start=(a == 0), stop=(a == 4))
            nc.scalar.activation(out=g2_s[:], in_=ph2[:], func=AF.Silu,
                                 bias=hb_s[:, 17:18], scale=1.0)
            del ph2
            uc = ucpool.tile([128, H], F32, tag="uc")
            nc.scalar.activation(out=uc[:], in_=g1_s[:], func=AF.Silu,
                                 bias=hb_s[:, 0:1], scale=float(a_ch[0]))
            nc.vector.tensor_scalar(out=acc_s[:], in0=uc[:],
                                    scalar1=float(fw[0]), scalar2=None,
                                    op0=OP.mult)
            for ch in range(1, 16):
                src = g1_s if ch < 8 else g2_s
                uc = ucpool.tile([128, H], F32, tag="uc")
                nc.scalar.activation(out=uc[:], in_=src[:], func=AF.Silu,
                                     bias=hb_s[:, ch:ch + 1],
                                     scale=float(a_ch[ch]))
                nc.vector.scalar_tensor_tensor(
                    out=acc_s[:], in0=uc[:], scalar=float(fw[ch]),
                    in1=acc_s[:], op0=OP.mult, op1=OP.add)
            nc.scalar.activation(out=acc_s[:], in_=acc_s[:], func=AF.Tanh,
                                 bias=hb_s[:, 18:19], scale=0.5)
            nc.vector.tensor_scalar(out=attnv_s[:], in0=acc_s[:],
                                    scalar1=gamma / 2.0,
                                    scalar2=1.0 + gamma / 2.0,
                                    op0=OP.mult, op1=OP.add)
            psel = ps_s.tile([128, 64], F32, tag="sm", name="psel")
            nc.tensor.matmul(psel[:, :], attnv_s[:], selm_s[:],
                             start=True, stop=True)
            nc.vector.tensor_copy(out=attnvh_s[:], in_=psel[:])
            del psel

            # ================= final =================
            dmaq = [nc.sync, nc.scalar, nc.gpsimd, nc.scalar,
                    nc.sync, nc.scalar, nc.gpsimd, nc.sync,
                    nc.scalar, nc.gpsimd, nc.sync, nc.scalar,
                    nc.sync, nc.gpsimd, nc.scalar, nc.sync]
            pool_tt = {2, 5, 8, 11}
            for i in range(16):
                c0 = 16 * i
                ot = opool.tile([128, 16, 64], BF16, tag="out",
                                name=f"ot{i}")
                av = attnvh_s[:]
                a_b = bass.AP(tensor=av.tensor, offset=av.offset,
                              ap=[list(av.ap[0]), [0, 16], [1, 64]])
                eng = nc.gpsimd if i in pool_tt else nc.vector
                eng.tensor_tensor(out=ot[:], in0=xwm_s[:, c0:c0 + 16, :],
                                  in1=a_b, op=OP.mult)
                dmaq[i].dma_start(out=outd.ap()[:, c0:c0 + 16, :],
                                  in_=ot[:])

    nc.compile()
    return nc


def _shard_inputs(inp, consts):
    x = np.asarray(inp["x"], np.float32)
    in_maps = []
    for j in range(NCORES):
        b, half = j // 2, j % 2
        r0 = 64 * half
        xb = x[b]
        xf8 = np.ascontiguousarray(
            xb.reshape(2, 128, H, W).transpose(1, 0, 2, 3)).astype(NPFP8)
        xwm = np.ascontiguousarray(
            xb[:, r0:r0 + 64, :].transpose(2, 0, 1)).astype(NPBF16)
        selm = np.zeros((128, 64), NPBF16)
        selm[r0 + np.arange(64), np.arange(64)] = 1
        in_maps.append({
            "xf8d": xf8, "xwmd": xwm, "selmd": selm,
            "w1d": consts["w1d"], "t1d": consts["t1"],
            "dwvd": consts["dwv"], "t2d": consts["t2"],
            "drad": consts["drhs_a"], "drbd": consts["drhs_b"],
            "pk2d": consts["pk2"],
            "b3wd": consts["b3w"], "b5wd": consts["b5w"],
            "hbd": consts["hb"], "idmd": consts["idm"],
        })
    return in_maps


def kernel(**inputs) -> np.ndarray:
    consts = _prep_consts(inputs)
    key = "nc"
    if key not in _CACHE:
        _CACHE[key] = _build_kernel(consts)
    nc = _CACHE[key]
    in_maps = _shard_inputs(inputs, consts)
    res = run_bass_kernel_spmd(nc, in_maps, list(range(NCORES)))
    out = np.empty((B, C, H, W), np.float32)
    for j in range(NCORES):
        b, half = j // 2, j % 2
        shard = np.asarray(res.results[j]["outd"]).astype(np.float32)
        out[b, :, 64 * half:64 * half + 64, :] = shard.transpose(1, 2, 0)
    return out
